# revision 4
# baseline (speedup 1.0000x reference)
"""Sliding-window GQA attention on 8 TRN2 NeuronCores.

Sharding: core c handles batch b=c//4 and kv-head pair 2*(c%4)..+1
(-> 4 query heads, 2 kv heads, all 2048 tokens of one batch).
Each core computes its heads' partial o-projection [2048, 3584];
the host sums the 4 partials per batch. No on-device collectives.

All heavy matmuls run in bf16 (fp32 PSUM accumulate). Attention is
computed fully transposed (logits^T[s,t] = k^T-stationary @ q^T):
exp() writes probs^T straight to SBUF where it is the PV stationary
operand, PV emits out^T[h,t] which feeds the o-projection directly,
and the softmax denominator comes from a ones-vector matmul group.
No PE transposes or PSUM->SBUF prob copies anywhere in phase 2.
SCALE/rms(q) and 1/rms(k) are folded into qT/kT during phase 1.
"""

import os
import numpy as np
import ml_dtypes

B, T, D, H = 2, 2048, 3584, 256
QH, KVH = 4, 2          # per-core q heads / kv heads
DC = D // 128           # 28 contract chunks
TBN = T // 128          # 16 token blocks
HC = H // 128           # 2 head-dim chunks
OC = QH * H // 128      # 8 out-proj contract chunks
SCALE = 0.0625
EPS = 1e-6
ROPE_BASE = 10000.0
WB = 1024 // 128        # window in blocks (8)
NEG = -1.0e30
NDOUT = D // 512        # 7 o-proj column chunks

BF16 = ml_dtypes.bfloat16

_cached = {}


def _build():
    import concourse.bass as bass
    import concourse.mybir as mybir
    import concourse.tile as tile
    from concourse import bacc

    f32 = mybir.dt.float32
    bf16 = mybir.dt.bfloat16
    AF = mybir.ActivationFunctionType

    nc = bacc.Bacc(None, target_bir_lowering=False)

    xT_d = nc.dram_tensor("xT", [128, DC, T], bf16, kind="ExternalInput")
    wq_d = nc.dram_tensor("wq", [128, DC, QH * H], bf16, kind="ExternalInput")
    wkv_d = nc.dram_tensor("wkv", [128, DC, 2 * KVH * H], bf16, kind="ExternalInput")
    wo_d = nc.dram_tensor("wo", [128, OC, D], bf16, kind="ExternalInput")
    cos_d = nc.dram_tensor("cos", [128, TBN, 128], f32, kind="ExternalInput")
    sin_d = nc.dram_tensor("sin", [128, TBN, 128], f32, kind="ExternalInput")
    qsc_d = nc.dram_tensor("qsc", [128, HC], f32, kind="ExternalInput")
    ksc_d = nc.dram_tensor("ksc", [128, HC], f32, kind="ExternalInput")
    mdiag_d = nc.dram_tensor("mdiag", [128, 128], f32, kind="ExternalInput")
    medge_d = nc.dram_tensor("medge", [128, 128], f32, kind="ExternalInput")
    out_d = nc.dram_tensor("out", [T, D], f32, kind="ExternalOutput")

    with tile.TileContext(nc) as tc:
        with (
            tc.tile_pool(name="persist", bufs=1) as pers,
            tc.tile_pool(name="wpool", bufs=6) as wpool,
        ):
            qT = pers.tile([128, HC, QH, T], bf16)     # q^T  [h, hc, head, t]
            kT = pers.tile([128, HC, KVH, T], bf16)    # k^T  [h, hc, kv, s]
            vS = pers.tile([128, TBN, KVH, H], bf16)   # v    [s, sblock, kv, h]
            ident = pers.tile([128, 128], bf16)
            from concourse.masks import make_identity
            make_identity(nc, ident)
            ones = pers.tile([128, 128], bf16)
            nc.gpsimd.memset(ones, 1.0)
            epsb = pers.tile([128, 1], f32)
            nc.gpsimd.memset(epsb, EPS)
            epsb2 = pers.tile([128, 1], f32)
            nc.gpsimd.memset(epsb2, EPS / (SCALE * SCALE))
            mdiagT = pers.tile([128, 128], f32)
            medgeT = pers.tile([128, 128], f32)

            # ---------------- phase 1: projections ----------------
            with (
                tc.tile_pool(name="tabs", bufs=1) as ptab,
                tc.tile_pool(name="xt", bufs=3) as pxt,
                tc.tile_pool(name="scr", bufs=3) as scr,
                tc.tile_pool(name="ppq", bufs=6, space=bass.MemorySpace.PSUM) as ppq,
                tc.tile_pool(name="ptr", bufs=2, space=bass.MemorySpace.PSUM) as ptrp,
            ):
                # wq arrives over BOTH hardware DGE queues (sync + scalar),
                # interleaved with xt0 so the first tb's matmuls start ASAP.
                WSPLIT = (4, 8, 8, 8)
                WOFF = (0, 4, 12, 20)
                wq_p = [wpool.tile([128, WSPLIT[qi], QH * H], bf16, tag="w",
                                   name=f"wq_{qi}") for qi in range(4)]
                xt0 = pxt.tile([128, DC, 128], bf16, tag="xt", name="xt0")
                nc.sync.dma_start(xt0, xT_d[:, :, 0:128])
                nc.scalar.dma_start(wq_p[0], wq_d[:, 0:4, :])
                nc.sync.dma_start(wq_p[1], wq_d[:, 4:12, :])
                nc.scalar.dma_start(wq_p[2], wq_d[:, 12:20, :])
                nc.sync.dma_start(wq_p[3], wq_d[:, 20:28, :])
                nc.scalar.dma_start(mdiagT, mdiag_d[:])
                nc.scalar.dma_start(medgeT, medge_d[:])

                xt1 = pxt.tile([128, DC, 128], bf16, tag="xt", name="xt1")
                nc.gpsimd.dma_start(xt1, xT_d[:, :, 128:256])
                xt2 = pxt.tile([128, DC, 128], bf16, tag="xt", name="xt2")
                nc.gpsimd.dma_start(xt2, xT_d[:, :, 256:384])
                cost = ptab.tile([128, TBN, 128], f32)
                sint = ptab.tile([128, TBN, 128], f32)
                qsc = ptab.tile([128, HC], f32)
                ksc = ptab.tile([128, HC], f32)
                nc.gpsimd.dma_start(cost, cos_d[:])
                nc.gpsimd.dma_start(sint, sin_d[:])
                nc.gpsimd.dma_start(qsc, qsc_d[:])
                nc.gpsimd.dma_start(ksc, ksc_d[:])

                def proj_epilogue(pq, j, tb, scv, dstT, slot, qnorm=False):
                    """norm+rope head j of psum pq -> transpose into
                    dstT[:, hc, slot, tb]. 1/rms (x SCALE for q) is folded in;
                    the QK-norm per-h scale is applied post-transpose."""
                    sq = scr.tile([128, H], f32, tag="sq")
                    ssq = scr.tile([128, 1], f32, tag="ssq")
                    nc.scalar.activation(sq, pq[:, j, :], AF.Square, accum_out=ssq)
                    std = scr.tile([128, 1], f32, tag="std")
                    if qnorm:
                        nc.scalar.activation(std, ssq, AF.Sqrt, bias=epsb2[:, 0:1],
                                             scale=1.0 / (H * SCALE * SCALE))
                    else:
                        nc.scalar.activation(std, ssq, AF.Sqrt, bias=epsb[:, 0:1],
                                             scale=1.0 / H)
                    rstd = scr.tile([128, 1], f32, tag="rstd")
                    nc.vector.reciprocal(rstd, std)
                    rb = rstd[:, 0:1].to_broadcast((128, 128))
                    x1 = pq[:, j, 0:128]
                    x2 = pq[:, j, 128:256]
                    t1 = scr.tile([128, 128], f32, tag="t1")
                    qr = scr.tile([128, H], bf16, tag="qr")
                    t2 = scr.tile([128, 128], f32, tag="t2")
                    nc.vector.tensor_mul(t1, x1, cost[:, tb, :])
                    nc.vector.tensor_mul(t2, x2, sint[:, tb, :])
                    nc.vector.tensor_sub(t1, t1, t2)
                    nc.vector.tensor_mul(qr[:, 0:128], t1, rb)
                    nc.vector.tensor_mul(t1, x2, cost[:, tb, :])
                    nc.vector.tensor_mul(t2, x1, sint[:, tb, :])
                    nc.vector.tensor_add(t1, t1, t2)
                    nc.vector.tensor_mul(qr[:, 128:256], t1, rb)
                    for hc in range(HC):
                        ptr = ptrp.tile([128, 128], bf16, tag="ptr")
                        nc.tensor.transpose(ptr, qr[:, hc * 128:(hc + 1) * 128], ident)
                        nc.vector.tensor_mul(
                            dstT[:, hc, slot, tb * 128:(tb + 1) * 128], ptr,
                            scv[:, hc:hc + 1].to_broadcast((128, 128)))

                def proj_block(tb, w_p, xt=None, xoff=0, woff=None):
                    if xt is None:
                        xt = pxt.tile([128, DC, 128], bf16, tag="xt")
                        nc.gpsimd.dma_start(xt, xT_d[:, :, tb * 128:(tb + 1) * 128])
                        xoff = 0
                    def wsel(dc):
                        if woff is None:
                            return w_p[dc // (DC // 4)], dc % (DC // 4)
                        for qi in range(3, -1, -1):
                            if dc >= woff[qi]:
                                return w_p[qi], dc - woff[qi]
                    pqa = ppq.tile([128, 2, H], f32, tag="pq", name="pqa")
                    pqb = ppq.tile([128, 2, H], f32, tag="pq", name="pqb")
                    for dc in range(DC):
                        lhsT = xt[:, dc, xoff:xoff + 128]
                        wt, dcl = wsel(dc)
                        nc.tensor.matmul(pqa[:, :, :], lhsT, wt[:, dcl, 0:512],
                                         start=(dc == 0), stop=(dc == DC - 1))
                        nc.tensor.matmul(pqb[:, :, :], lhsT, wt[:, dcl, 512:1024],
                                         start=(dc == 0), stop=(dc == DC - 1))
                    return (pqa, pqb)

                # --- 1a: Q ---
                wkv_p = None
                for tb in range(TBN):
                    if tb == 0:
                        pq2 = proj_block(tb, wq_p, xt=xt0, woff=WOFF)
                    elif tb in (1, 2):
                        pq2 = proj_block(tb, wq_p, xt=(xt1 if tb == 1 else xt2),
                                         woff=WOFF)
                    else:
                        pq2 = proj_block(tb, wq_p, woff=WOFF)
                    for j in range(QH):
                        proj_epilogue(pq2[j // 2], j % 2, tb, qsc, qT, j,
                                      qnorm=True)
                    if tb == 7:
                        # paced prefetch on the gpsimd queue mid-phase
                        wkv_p = []
                        for qi in range(4):
                            wt = wpool.tile([128, DC // 4, 2 * KVH * H], bf16,
                                            tag="w", name=f"wkv_{qi}")
                            nc.gpsimd.dma_start(
                                wt, wkv_d[:, qi * (DC // 4):(qi + 1) * (DC // 4), :])
                            wkv_p.append(wt)

                # --- 1b: K and V ---
                for tb in range(TBN):
                    pq2 = proj_block(tb, wkv_p)
                    for kv in range(KVH):
                        proj_epilogue(pq2[0], kv, tb, ksc, kT, kv)
                    for kv in range(KVH):
                        nc.vector.tensor_copy(vS[:, tb, kv, :], pq2[1][:, kv, :])
                    if tb == 7:
                        wo_p = []
                        for qi in range(4):
                            wt = wpool.tile([128, OC // 4, D], bf16, tag="w",
                                            name=f"wo_{qi}")
                            nc.gpsimd.dma_start(
                                wt, wo_d[:, qi * (OC // 4):(qi + 1) * (OC // 4), :])
                            wo_p.append(wt)

            # ---------------- phase 2: attention + o-proj (transposed) -------
            with (
                tc.tile_pool(name="att", bufs=2) as att,
                tc.tile_pool(name="ysb", bufs=1) as pys,
                tc.tile_pool(name="pl", bufs=3, space=bass.MemorySpace.PSUM) as plp,
                tc.tile_pool(name="poa", bufs=1, space=bass.MemorySpace.PSUM) as poa,
                tc.tile_pool(name="pob", bufs=1, space=bass.MemorySpace.PSUM) as pob,
                tc.tile_pool(name="py", bufs=2, space=bass.MemorySpace.PSUM) as pyp,
            ):
                ysb = pys.tile([128, D], f32, tag="y")
                for tb in range(TBN):
                    sb0 = max(0, tb - WB)
                    ns = tb - sb0 + 1
                    outT = att.tile([128, OC, 128], bf16, tag="outT")
                    for j in range(QH):
                        kv = j // 2
                        pA = poa.tile([128, 512], f32, tag="poa")
                        pB = pob.tile([128, 512], f32, tag="pob")
                        pTs = []
                        for si in range(ns):
                            s = sb0 + si
                            pl = plp.tile([128, 512], f32, tag="pl", name="pl")
                            for hc in range(HC):
                                nc.tensor.matmul(
                                    pl[:, 0:128],
                                    kT[:, hc, kv, s * 128:(s + 1) * 128],
                                    qT[:, hc, j, tb * 128:(tb + 1) * 128],
                                    start=(hc == 0), stop=(hc == HC - 1))
                            if s == tb:
                                nc.vector.tensor_add(pl[:, 0:128], pl[:, 0:128],
                                                     mdiagT)
                            if tb >= WB and si == 0:
                                nc.vector.tensor_add(pl[:, 0:128], pl[:, 0:128],
                                                     medgeT)
                            pT = att.tile([128, 128], bf16, tag="pT", bufs=14,
                                          name="pT")
                            nc.scalar.activation(pT, pl[:, 0:128], AF.Exp)
                            pTs.append(pT)
                            nc.tensor.matmul(pA[:, 0:128], vS[:, s, kv, 0:128],
                                             pT, start=(si == 0),
                                             stop=(si == ns - 1))
                            nc.tensor.matmul(pB[:, 0:128], vS[:, s, kv, 128:256],
                                             pT, start=(si == 0),
                                             stop=(si == ns - 1))
                        pden = plp.tile([128, 512], f32, tag="pl", name="pden")
                        for si in range(ns):
                            nc.tensor.matmul(pden[:, 0:128], ones, pTs[si],
                                             start=(si == 0), stop=(si == ns - 1))
                        recipb = att.tile([128, 128], f32, tag="recip")
                        nc.vector.reciprocal(recipb, pden[:, 0:128])
                        nc.vector.tensor_mul(outT[:, 2 * j, :], pA[:, 0:128],
                                             recipb)
                        nc.vector.tensor_mul(outT[:, 2 * j + 1, :], pB[:, 0:128],
                                             recipb)
                    for dx in range(NDOUT):
                        py = pyp.tile([128, 512], f32, tag="py")
                        for c in range(OC):
                            nc.tensor.matmul(py, outT[:, c, :],
                                             wo_p[c // 2][:, c % 2,
                                                          dx * 512:(dx + 1) * 512],
                                             start=(c == 0), stop=(c == OC - 1))
                        if dx % 2 == 0:
                            nc.vector.tensor_copy(ysb[:, dx * 512:(dx + 1) * 512], py)
                        else:
                            nc.scalar.activation(ysb[:, dx * 512:(dx + 1) * 512], py,
                                                 AF.Copy)
                        nc.sync.dma_start(
                            out_d[tb * 128:(tb + 1) * 128, dx * 512:(dx + 1) * 512],
                            ysb[:, dx * 512:(dx + 1) * 512])

    nc.compile()
    return nc


def _tile128(a):
    """[128*n, m] -> [128, n, m] with row index = chunk*128 + partition."""
    n = a.shape[0] // 128
    return np.ascontiguousarray(
        a.reshape(n, 128, *a.shape[1:]).transpose(1, 0, *range(2, a.ndim + 1)))


def _rope_tabs():
    j = np.arange(128, dtype=np.float64)
    ts = ROPE_BASE ** (2.0 * j / H)
    ang = np.arange(T, dtype=np.float64)[:, None] / ts[None, :]
    return (_tile128(np.cos(ang).astype(np.float32)),
            _tile128(np.sin(ang).astype(np.float32)))


def kernel(x, w_q, w_kv, w_o, q_norm_scale, k_norm_scale):
    from concourse.bass_utils import run_bass_kernel_spmd

    if "nc" not in _cached:
        _cached["nc"] = _build()
    nc = _cached["nc"]

    x = np.asarray(x, np.float32)
    w_q = np.asarray(w_q, np.float32)
    w_kv = np.asarray(w_kv, np.float32)
    w_o = np.asarray(w_o, np.float32)
    cos_t, sin_t = _rope_tabs()
    qsc = np.ascontiguousarray(
        np.asarray(q_norm_scale, np.float32).reshape(HC, 128).T)
    ksc = np.ascontiguousarray(
        np.asarray(k_norm_scale, np.float32).reshape(HC, 128).T)

    p = np.arange(128)[:, None]   # s within block (partitions)
    f = np.arange(128)[None, :]   # t within block (free)
    mdiagT = np.where(p <= f, 0.0, NEG).astype(np.float32)
    medgeT = np.where(p >= f + 1, 0.0, NEG).astype(np.float32)

    xT_b = []
    for b in range(B):
        xT_b.append(_tile128(np.ascontiguousarray(x[b].T).astype(BF16)))

    in_maps = []
    for c in range(8):
        b, kp = c // 4, c % 4
        n0, k0 = 4 * kp, 2 * kp
        wq = _tile128(w_q[n0:n0 + 4].transpose(1, 0, 2).reshape(D, QH * H).astype(BF16))
        wk = w_kv[0, k0:k0 + 2].transpose(1, 0, 2).reshape(D, KVH * H)
        wv = w_kv[1, k0:k0 + 2].transpose(1, 0, 2).reshape(D, KVH * H)
        wkv = _tile128(np.concatenate([wk, wv], axis=1).astype(BF16))
        wo = _tile128(w_o[n0:n0 + 4].reshape(QH * H, D).astype(BF16))
        m = {"xT": xT_b[b], "wq": wq, "wkv": wkv, "wo": wo,
             "mdiag": mdiagT, "medge": medgeT,
             "cos": cos_t, "sin": sin_t, "qsc": qsc, "ksc": ksc}
        in_maps.append(m)

    res = run_bass_kernel_spmd(nc, in_maps, core_ids=list(range(8)))
    _cached["last_result"] = res
    y = np.zeros((B, T, D), np.float32)
    for c in range(8):
        y[c // 4] += np.asarray(res.results[c]["out"], np.float32)
    return y


# revision 11
# speedup vs baseline: 1.0176x; 1.0176x over previous
"""Sliding-window GQA attention on 8 TRN2 NeuronCores.

Sharding: core c handles batch b=c//4 and kv-head pair 2*(c%4)..+1
(-> 4 query heads, 2 kv heads, all 2048 tokens of one batch).
Each core computes its heads' partial o-projection [2048, 3584];
the host sums the 4 partials per batch. No on-device collectives.

All heavy matmuls run in bf16 (fp32 PSUM accumulate). Attention is
computed fully transposed (logits^T[s,t] = k^T-stationary @ q^T):
exp() writes probs^T straight to SBUF where it is the PV stationary
operand, PV emits out^T[h,t] which feeds the o-projection directly,
and the softmax denominator comes from a ones-vector matmul group.
No PE transposes or PSUM->SBUF prob copies anywhere in phase 2.
SCALE/rms(q) and 1/rms(k) are folded into qT/kT during phase 1.
"""

import os
import numpy as np
import ml_dtypes

B, T, D, H = 2, 2048, 3584, 256
QH, KVH = 4, 2          # per-core q heads / kv heads
DC = D // 128           # 28 contract chunks
TBN = T // 128          # 16 token blocks
HC = H // 128           # 2 head-dim chunks
OC = QH * H // 128      # 8 out-proj contract chunks
SCALE = 0.0625
EPS = 1e-6
ROPE_BASE = 10000.0
WB = 1024 // 128        # window in blocks (8)
NEG = -1.0e30
NDOUT = D // 512        # 7 o-proj column chunks

BF16 = ml_dtypes.bfloat16

_cached = {}


def _build():
    import concourse.bass as bass
    import concourse.mybir as mybir
    import concourse.tile as tile
    from concourse import bacc

    f32 = mybir.dt.float32
    bf16 = mybir.dt.bfloat16
    AF = mybir.ActivationFunctionType

    nc = bacc.Bacc(None, target_bir_lowering=False)

    xT_d = nc.dram_tensor("xT", [128, TBN, DC, 128], bf16, kind="ExternalInput")
    wq_d = nc.dram_tensor("wq", [128, DC, QH * H], bf16, kind="ExternalInput")
    wkv_d = nc.dram_tensor("wkv", [128, DC, 2 * KVH * H], bf16, kind="ExternalInput")
    wo_d = nc.dram_tensor("wo", [128, OC, D], bf16, kind="ExternalInput")
    cos_d = nc.dram_tensor("cos", [128, TBN, 128], f32, kind="ExternalInput")
    sin_d = nc.dram_tensor("sin", [128, TBN, 128], f32, kind="ExternalInput")
    qsc_d = nc.dram_tensor("qsc", [128, HC], f32, kind="ExternalInput")
    ksc_d = nc.dram_tensor("ksc", [128, HC], f32, kind="ExternalInput")
    mdiag_d = nc.dram_tensor("mdiag", [128, 128], f32, kind="ExternalInput")
    medge_d = nc.dram_tensor("medge", [128, 128], f32, kind="ExternalInput")
    out_d = nc.dram_tensor("out", [T, D], f32, kind="ExternalOutput")

    with tile.TileContext(nc) as tc:
        with (
            tc.tile_pool(name="persist", bufs=1) as pers,
            tc.tile_pool(name="wpool", bufs=6) as wpool,
        ):
            qT = pers.tile([128, HC, QH, T], bf16)     # q^T  [h, hc, head, t]
            kT = pers.tile([128, HC, KVH, T], bf16)    # k^T  [h, hc, kv, s]
            vS = pers.tile([128, TBN, KVH, H], bf16)   # v    [s, sblock, kv, h]
            ident = pers.tile([128, 128], bf16)
            from concourse.masks import make_identity
            make_identity(nc, ident)
            ones = pers.tile([128, 128], bf16)
            nc.gpsimd.memset(ones, 1.0)
            epsb = pers.tile([128, 1], f32)
            nc.gpsimd.memset(epsb, EPS)
            epsb2 = pers.tile([128, 1], f32)
            nc.gpsimd.memset(epsb2, EPS / (SCALE * SCALE))
            mdiagT = pers.tile([128, 128], f32)
            medgeT = pers.tile([128, 128], f32)

            # ---------------- phase 1: projections ----------------
            with (
                tc.tile_pool(name="tabs", bufs=1) as ptab,
                tc.tile_pool(name="xt", bufs=3) as pxt,
                tc.tile_pool(name="scr", bufs=3) as scr,
                tc.tile_pool(name="ppq", bufs=6, space=bass.MemorySpace.PSUM) as ppq,
                tc.tile_pool(name="ptr", bufs=2, space=bass.MemorySpace.PSUM) as ptrp,
            ):
                # wq arrives over BOTH hardware DGE queues (sync + scalar),
                # interleaved with xt0 so the first tb's matmuls start ASAP.
                WSPLIT = (4, 8, 8, 8)
                WOFF = (0, 4, 12, 20)
                wq_p = [wpool.tile([128, WSPLIT[qi], QH * H], bf16, tag="w",
                                   name=f"wq_{qi}") for qi in range(4)]
                xt0 = pxt.tile([128, DC, 128], bf16, tag="xt", name="xt0")
                nc.scalar.dma_start(xt0, xT_d[:, 0, :, :])
                nc.scalar.dma_start(wq_p[0], wq_d[:, 0:4, :])
                nc.sync.dma_start(wq_p[1], wq_d[:, 4:12, :])
                nc.scalar.dma_start(wq_p[2], wq_d[:, 12:20, :])
                nc.sync.dma_start(wq_p[3], wq_d[:, 20:28, :])
                nc.scalar.dma_start(mdiagT, mdiag_d[:])
                nc.scalar.dma_start(medgeT, medge_d[:])

                xt1 = pxt.tile([128, DC, 128], bf16, tag="xt", name="xt1")
                nc.gpsimd.dma_start(xt1, xT_d[:, 1, :, :])
                xt2 = pxt.tile([128, DC, 128], bf16, tag="xt", name="xt2")
                nc.gpsimd.dma_start(xt2, xT_d[:, 2, :, :])
                cost = ptab.tile([128, TBN, 128], f32)
                sint = ptab.tile([128, TBN, 128], f32)
                qsc = ptab.tile([128, HC], f32)
                ksc = ptab.tile([128, HC], f32)
                nc.gpsimd.dma_start(cost, cos_d[:])
                nc.gpsimd.dma_start(sint, sin_d[:])
                nc.gpsimd.dma_start(qsc, qsc_d[:])
                nc.gpsimd.dma_start(ksc, ksc_d[:])

                def proj_epilogue(pq, j, tb, scv, dstT, slot, qnorm=False):
                    """norm+rope head j of psum pq -> transpose into
                    dstT[:, hc, slot, tb]. 1/rms (x SCALE for q) is folded in;
                    the QK-norm per-h scale is applied post-transpose."""
                    sq = scr.tile([128, H], f32, tag="sq")
                    ssq = scr.tile([128, 1], f32, tag="ssq")
                    nc.scalar.activation(sq, pq[:, j, :], AF.Square, accum_out=ssq)
                    std = scr.tile([128, 1], f32, tag="std")
                    if qnorm:
                        nc.scalar.activation(std, ssq, AF.Sqrt, bias=epsb2[:, 0:1],
                                             scale=1.0 / (H * SCALE * SCALE))
                    else:
                        nc.scalar.activation(std, ssq, AF.Sqrt, bias=epsb[:, 0:1],
                                             scale=1.0 / H)
                    rstd = scr.tile([128, 1], f32, tag="rstd")
                    nc.vector.reciprocal(rstd, std)
                    rb = rstd[:, 0:1].to_broadcast((128, 128))
                    x1 = pq[:, j, 0:128]
                    x2 = pq[:, j, 128:256]
                    t1 = scr.tile([128, 128], f32, tag="t1")
                    qr = scr.tile([128, H], bf16, tag="qr")
                    t2 = scr.tile([128, 128], f32, tag="t2")
                    nc.vector.tensor_mul(t1, x1, cost[:, tb, :])
                    nc.vector.tensor_mul(t2, x2, sint[:, tb, :])
                    nc.vector.tensor_sub(t1, t1, t2)
                    nc.vector.tensor_mul(qr[:, 0:128], t1, rb)
                    nc.vector.tensor_mul(t1, x2, cost[:, tb, :])
                    nc.vector.tensor_mul(t2, x1, sint[:, tb, :])
                    nc.vector.tensor_add(t1, t1, t2)
                    nc.vector.tensor_mul(qr[:, 128:256], t1, rb)
                    for hc in range(HC):
                        ptr = ptrp.tile([128, 128], bf16, tag="ptr")
                        nc.tensor.transpose(ptr, qr[:, hc * 128:(hc + 1) * 128], ident)
                        nc.vector.tensor_mul(
                            dstT[:, hc, slot, tb * 128:(tb + 1) * 128], ptr,
                            scv[:, hc:hc + 1].to_broadcast((128, 128)))

                def proj_block(tb, w_p, xt=None, xoff=0, woff=None):
                    if xt is None:
                        xt = pxt.tile([128, DC, 128], bf16, tag="xt")
                        nc.gpsimd.dma_start(xt, xT_d[:, tb, :, :])
                        xoff = 0
                    def wsel(dc):
                        if woff is None:
                            return w_p[dc // (DC // 4)], dc % (DC // 4)
                        for qi in range(3, -1, -1):
                            if dc >= woff[qi]:
                                return w_p[qi], dc - woff[qi]
                    pqa = ppq.tile([128, 2, H], f32, tag="pq", name="pqa")
                    pqb = ppq.tile([128, 2, H], f32, tag="pq", name="pqb")
                    for dc in range(DC):
                        lhsT = xt[:, dc, xoff:xoff + 128]
                        wt, dcl = wsel(dc)
                        nc.tensor.matmul(pqa[:, :, :], lhsT, wt[:, dcl, 0:512],
                                         start=(dc == 0), stop=(dc == DC - 1))
                        nc.tensor.matmul(pqb[:, :, :], lhsT, wt[:, dcl, 512:1024],
                                         start=(dc == 0), stop=(dc == DC - 1))
                    return (pqa, pqb)

                # --- 1a: Q ---
                wkv_p = None
                for tb in range(TBN):
                    if tb == 0:
                        pq2 = proj_block(tb, wq_p, xt=xt0, woff=WOFF)
                    elif tb in (1, 2):
                        pq2 = proj_block(tb, wq_p, xt=(xt1 if tb == 1 else xt2),
                                         woff=WOFF)
                    else:
                        pq2 = proj_block(tb, wq_p, woff=WOFF)
                    for j in range(QH):
                        proj_epilogue(pq2[j // 2], j % 2, tb, qsc, qT, j,
                                      qnorm=True)
                    if tb == 7:
                        # paced prefetch on the gpsimd queue mid-phase
                        wkv_p = []
                        for qi in range(4):
                            wt = wpool.tile([128, DC // 4, 2 * KVH * H], bf16,
                                            tag="w", name=f"wkv_{qi}")
                            nc.gpsimd.dma_start(
                                wt, wkv_d[:, qi * (DC // 4):(qi + 1) * (DC // 4), :])
                            wkv_p.append(wt)

                # --- 1b: K and V ---
                for tb in range(TBN):
                    pq2 = proj_block(tb, wkv_p)
                    for kv in range(KVH):
                        proj_epilogue(pq2[0], kv, tb, ksc, kT, kv)
                    for kv in range(KVH):
                        nc.vector.tensor_copy(vS[:, tb, kv, :], pq2[1][:, kv, :])
                    if tb == 7:
                        wo_p = []
                        for qi in range(4):
                            wt = wpool.tile([128, OC // 4, D], bf16, tag="w",
                                            name=f"wo_{qi}")
                            nc.gpsimd.dma_start(
                                wt, wo_d[:, qi * (OC // 4):(qi + 1) * (OC // 4), :])
                            wo_p.append(wt)

            # ---------------- phase 2: attention + o-proj (transposed) -------
            with (
                tc.tile_pool(name="att", bufs=2) as att,
                tc.tile_pool(name="ysb", bufs=1) as pys,
                tc.tile_pool(name="pl", bufs=3, space=bass.MemorySpace.PSUM) as plp,
                tc.tile_pool(name="poa", bufs=2, space=bass.MemorySpace.PSUM) as poa,
                tc.tile_pool(name="pob", bufs=1, space=bass.MemorySpace.PSUM) as pob,
                tc.tile_pool(name="py", bufs=2, space=bass.MemorySpace.PSUM) as pyp,
            ):
                ysb = pys.tile([128, D], f32, tag="y")
                for tb in range(TBN):
                    sb0 = max(0, tb - WB)
                    ns = tb - sb0 + 1
                    outT = att.tile([128, OC, 128], bf16, tag="outT")
                    for j in range(QH):
                        kv = j // 2
                        pA = poa.tile([128, 512], f32, tag="poa")
                        pB = pob.tile([128, 512], f32, tag="pob")
                        # den shares pB's bank: the single start=True (pB's
                        # si=0 PV matmul) clears the bank's has_written bits;
                        # den's si=0 matmul then runs with start=False and
                        # overwrites its (cleared) region, accumulating after.
                        for si in range(ns):
                            s = sb0 + si
                            pl = plp.tile([128, 512], f32, tag="pl", name="pl")
                            for hc in range(HC):
                                nc.tensor.matmul(
                                    pl[:, 0:128],
                                    kT[:, hc, kv, s * 128:(s + 1) * 128],
                                    qT[:, hc, j, tb * 128:(tb + 1) * 128],
                                    start=(hc == 0), stop=(hc == HC - 1))
                            if s == tb:
                                nc.vector.tensor_add(pl[:, 0:128], pl[:, 0:128],
                                                     mdiagT)
                            if tb >= WB and si == 0:
                                nc.vector.tensor_add(pl[:, 0:128], pl[:, 0:128],
                                                     medgeT)
                            pT = att.tile([128, 128], bf16, tag="pT", bufs=14,
                                          name="pT")
                            nc.scalar.activation(pT, pl[:, 0:128], AF.Exp)
                            nc.tensor.matmul(pA[:, 0:128], vS[:, s, kv, 0:128],
                                             pT, start=(si == 0),
                                             stop=(si == ns - 1))
                            nc.tensor.matmul(pB[:, 0:128], vS[:, s, kv, 128:256],
                                             pT, start=(si == 0),
                                             stop=(si == ns - 1),
                                             skip_group_check=True)
                            nc.tensor.matmul(pB[:, 128:256], ones, pT,
                                             start=False, stop=(si == ns - 1),
                                             skip_group_check=True)
                        recipb = att.tile([128, 128], f32, tag="recip")
                        nc.vector.reciprocal(recipb, pB[:, 128:256])
                        nc.vector.tensor_mul(outT[:, 2 * j, :], pA[:, 0:128],
                                             recipb)
                        nc.vector.tensor_mul(outT[:, 2 * j + 1, :], pB[:, 0:128],
                                             recipb)
                    for dx in range(NDOUT):
                        py = pyp.tile([128, 512], f32, tag="py")
                        for c in range(OC):
                            nc.tensor.matmul(py, outT[:, c, :],
                                             wo_p[c // 2][:, c % 2,
                                                          dx * 512:(dx + 1) * 512],
                                             start=(c == 0), stop=(c == OC - 1))
                        nc.vector.tensor_copy(ysb[:, dx * 512:(dx + 1) * 512], py)
                        nc.sync.dma_start(
                            out_d[tb * 128:(tb + 1) * 128, dx * 512:(dx + 1) * 512],
                            ysb[:, dx * 512:(dx + 1) * 512])

    nc.compile()
    return nc


def _tile128(a):
    """[128*n, m] -> [128, n, m] with row index = chunk*128 + partition."""
    n = a.shape[0] // 128
    return np.ascontiguousarray(
        a.reshape(n, 128, *a.shape[1:]).transpose(1, 0, *range(2, a.ndim + 1)))


def _rope_tabs():
    j = np.arange(128, dtype=np.float64)
    ts = ROPE_BASE ** (2.0 * j / H)
    ang = np.arange(T, dtype=np.float64)[:, None] / ts[None, :]
    return (_tile128(np.cos(ang).astype(np.float32)),
            _tile128(np.sin(ang).astype(np.float32)))


def kernel(x, w_q, w_kv, w_o, q_norm_scale, k_norm_scale):
    from concourse.bass_utils import run_bass_kernel_spmd

    if "nc" not in _cached:
        _cached["nc"] = _build()
    nc = _cached["nc"]

    x = np.asarray(x, np.float32)
    w_q = np.asarray(w_q, np.float32)
    w_kv = np.asarray(w_kv, np.float32)
    w_o = np.asarray(w_o, np.float32)
    cos_t, sin_t = _rope_tabs()
    qsc = np.ascontiguousarray(
        np.asarray(q_norm_scale, np.float32).reshape(HC, 128).T)
    ksc = np.ascontiguousarray(
        np.asarray(k_norm_scale, np.float32).reshape(HC, 128).T)

    p = np.arange(128)[:, None]   # s within block (partitions)
    f = np.arange(128)[None, :]   # t within block (free)
    mdiagT = np.where(p <= f, 0.0, NEG).astype(np.float32)
    medgeT = np.where(p >= f + 1, 0.0, NEG).astype(np.float32)

    xT_b = []
    for b in range(B):
        xt = _tile128(np.ascontiguousarray(x[b].T).astype(BF16))  # [128, DC, T]
        xt = np.ascontiguousarray(
            xt.reshape(128, DC, TBN, 128).transpose(0, 2, 1, 3))
        xT_b.append(xt)

    in_maps = []
    for c in range(8):
        b, kp = c // 4, c % 4
        n0, k0 = 4 * kp, 2 * kp
        wq = _tile128(w_q[n0:n0 + 4].transpose(1, 0, 2).reshape(D, QH * H).astype(BF16))
        wk = w_kv[0, k0:k0 + 2].transpose(1, 0, 2).reshape(D, KVH * H)
        wv = w_kv[1, k0:k0 + 2].transpose(1, 0, 2).reshape(D, KVH * H)
        wkv = _tile128(np.concatenate([wk, wv], axis=1).astype(BF16))
        wo = _tile128(w_o[n0:n0 + 4].reshape(QH * H, D).astype(BF16))
        m = {"xT": xT_b[b], "wq": wq, "wkv": wkv, "wo": wo,
             "mdiag": mdiagT, "medge": medgeT,
             "cos": cos_t, "sin": sin_t, "qsc": qsc, "ksc": ksc}
        in_maps.append(m)

    res = run_bass_kernel_spmd(nc, in_maps, core_ids=list(range(8)))
    _cached["last_result"] = res
    y = np.zeros((B, T, D), np.float32)
    for c in range(8):
        y[c // 4] += np.asarray(res.results[c]["out"], np.float32)
    return y
